# revision 1
# baseline (speedup 1.0000x reference)
"""Trainium2 Bass kernel for LinearMemoryAttention (B=1, S=4096, D=512, H=8, Dh=64).

Sharding: sequence-parallel over 8 cores (512 rows each), all heads local.
Cross-core causal state is resolved with one small AllGather of per-block
(sigma(k) outer v, sigma(k)) sums plus a per-core prefix mask.

Self-contained: hardcodes all shapes; builds/compiles the Bass program once.
"""

import numpy as np

import concourse.bass as bass
import concourse.bacc as bacc
import concourse.mybir as mybir
import concourse.tile as tile
from concourse.bass_utils import run_bass_kernel_spmd

F32 = mybir.dt.float32
F32R = mybir.dt.float32r
N_CORES = 8
S = 4096
D = 512
H = 8
DH = 64
HP = 66  # head width incl. denominator column (+1 even-pad for f32r)
S_BLK = S // N_CORES  # 512 rows per core
NCH = S_BLK // 128  # 4 chunks of 128
NHP = H // 2  # 4 head pairs
EPS = 1e-6
W = NHP * HP  # 264

_CACHE = {}


def _build():
    _ctx = {}
    Alu = mybir.AluOpType
    Act = mybir.ActivationFunctionType
    nc = bacc.Bacc("TRN2", target_bir_lowering=False, debug=False,
                   num_devices=N_CORES)

    hs_d = nc.dram_tensor("hs", [S_BLK, D], F32, kind="ExternalInput").ap()
    wq_d = nc.dram_tensor("wq", [D, D], F32, kind="ExternalInput").ap()
    wk_d = nc.dram_tensor("wk", [D, D], F32, kind="ExternalInput").ap()
    wv_d = nc.dram_tensor("wv", [D, D], F32, kind="ExternalInput").ap()
    wo_d = nc.dram_tensor("wo", [D, D], F32, kind="ExternalInput").ap()
    bqt_d = nc.dram_tensor("bqt", [128, NHP], F32, kind="ExternalInput").ap()
    bkt_d = nc.dram_tensor("bkt", [128, NHP], F32, kind="ExternalInput").ap()
    bk_d = nc.dram_tensor("bkr", [1, D], F32, kind="ExternalInput").ap()
    bv_d = nc.dram_tensor("bvr", [1, D], F32, kind="ExternalInput").ap()
    mz_d = nc.dram_tensor("mz", [128, W], F32, kind="ExternalInput").ap()
    pm_d = nc.dram_tensor("pmask", [128, N_CORES], F32, kind="ExternalInput").ap()
    y_d = nc.dram_tensor("y", [S_BLK, D], F32, kind="ExternalOutput").ap()

    with tile.TileContext(nc) as tc:
        with (
            tc.tile_pool(name="const", bufs=1) as cpool,
            tc.tile_pool(name="wpool", bufs=1) as wpool,
            tc.tile_pool(name="data", bufs=1) as dpool,
            tc.tile_pool(name="tmp", bufs=3) as tpool,
            tc.tile_pool(name="small", bufs=4) as spool,
            tc.tile_pool(name="dram", bufs=1, space="DRAM") as drpool,
        ):
            # ---- input DMAs: hs first (feeds the critical path) -----------
            hs_t = [dpool.tile([128, D], F32, name=f"hs{i}") for i in range(4)]
            for i in range(4):
                nc.sync.dma_start(hs_t[i][:], hs_d[i * 128:(i + 1) * 128, :])
            wk_t = [wpool.tile([128, D], F32R, name=f"wk{i}") for i in range(4)]
            wv_t = [wpool.tile([128, D], F32R, name=f"wv{i}") for i in range(4)]
            wq_t = [wpool.tile([128, D], F32R, name=f"wq{i}") for i in range(4)]
            wo_t = [wpool.tile([128, D], F32R, name=f"wo{i}") for i in range(4)]
            for i in range(4):
                sl = slice(i * 128, (i + 1) * 128)
                nc.scalar.dma_start(wk_t[i][:], wk_d.bitcast(F32R)[sl, :])
                nc.scalar.dma_start(wv_t[i][:], wv_d.bitcast(F32R)[sl, :])
            bkr = cpool.tile([1, D], F32R)
            nc.sync.dma_start(bkr[:], bk_d.bitcast(F32R)[:])
            bvr = cpool.tile([1, D], F32R)
            nc.sync.dma_start(bvr[:], bv_d.bitcast(F32R)[:])
            mz = cpool.tile([128, W], F32R)
            nc.sync.dma_start(mz[:], mz_d.bitcast(F32R)[:])
            pmask = cpool.tile([128, N_CORES], F32)
            nc.sync.dma_start(pmask[:], pm_d[:])
            bqt = cpool.tile([128, NHP], F32)
            nc.sync.dma_start(bqt[:], bqt_d[:])
            bkt = cpool.tile([128, NHP], F32)
            nc.sync.dma_start(bkt[:], bkt_d[:])
            for i in range(4):
                sl = slice(i * 128, (i + 1) * 128)
                nc.scalar.dma_start(wq_t[i][:], wq_d.bitcast(F32R)[sl, :])
            for i in range(4):
                sl = slice(i * 128, (i + 1) * 128)
                nc.scalar.dma_start(wo_t[i][:], wo_d.bitcast(F32R)[sl, :])

            # ---- constants -------------------------------------------------
            ones128 = cpool.tile([128, 128], F32)
            nc.gpsimd.memset(ones128[:], 1.0)
            ident = cpool.tile([128, 128], F32)
            nc.gpsimd.affine_select(ident[:], ones128[:], pattern=[[1, 128]],
                                    compare_op=Alu.is_equal, fill=0.0, base=0,
                                    channel_multiplier=-1)
            triu2 = cpool.tile([128, 256], F32)
            for half in range(2):
                nc.gpsimd.affine_select(
                    triu2[:, half * 128:(half + 1) * 128], ones128[:],
                    pattern=[[1, 128]], compare_op=Alu.is_ge, fill=0.0,
                    base=0, channel_multiplier=-1)
            ones1 = cpool.tile([1, 128], F32R)
            nc.gpsimd.memset(ones1.bitcast(F32)[:], 1.0)

            hsT = [dpool.tile([128, S_BLK], F32R, name=f"hsT{i}")
                   for i in range(4)]
            sk_sb = [dpool.tile([128, D], F32R, name=f"sk{i}") for i in range(4)]
            v_sb = [dpool.tile([128, H * HP], F32R, name=f"v{i}")
                    for i in range(4)]
            L_sb = [None] + [dpool.tile([128, W], F32, name=f"L{c}")
                             for c in range(1, NCH)]
            ball = dpool.tile([128, W], F32, name="ball")

            with tc.tile_pool(name="ps1", bufs=1, space="PSUM") as ps1:
                # ---- all transposes first: dense PE work, warms HAM -------
                for st in range(4):
                    for dt in range(4):
                        pst = ps1.tile([128, 128], F32, name="pstr", bufs=4)
                        nc.tensor.transpose(
                            pst[:], hs_t[st][:, dt * 128:(dt + 1) * 128],
                            ident[:])
                        nc.vector.tensor_copy(
                            hsT[dt][:, st * 128:(st + 1) * 128], pst[:])
                # ---- per-chunk: k/v proj -> elu -> U ----------------------
                for st in range(4):
                    psk = ps1.tile([128, D], F32, name="psbig", bufs=3)
                    for dt in range(4):
                        nc.tensor.matmul(
                            psk[:], hsT[dt][:, st * 128:(st + 1) * 128],
                            wk_t[dt][:], start=(dt == 0), stop=False)
                    nc.tensor.matmul(psk[:], ones1[:], bkr[:],
                                     start=False, stop=True)
                    e_t = tpool.tile([128, D], F32, name="elu_e")
                    r_t = tpool.tile([128, D], F32, name="elu_r")
                    nc.scalar.activation(e_t[:], psk[:], Act.Exp)
                    nc.scalar.activation(r_t[:], psk[:], Act.Relu)
                    nc.vector.scalar_tensor_tensor(
                        sk_sb[st][:], e_t[:], 1.0, r_t[:],
                        op0=Alu.min, op1=Alu.add)

                    psv = ps1.tile([128, D], F32, name="psbig", bufs=3)
                    for dt in range(4):
                        nc.tensor.matmul(
                            psv[:], hsT[dt][:, st * 128:(st + 1) * 128],
                            wv_t[dt][:], start=(dt == 0), stop=False)
                    nc.tensor.matmul(psv[:], ones1[:], bvr[:],
                                     start=False, stop=True)
                    v3 = v_sb[st].rearrange("p (h e) -> p h e", e=HP)
                    nc.vector.tensor_copy(
                        v3[:, :, 0:DH],
                        psv.rearrange("p (h e) -> p h e", e=DH))
                    nc.gpsimd.memset(v3.bitcast(F32)[:, :, DH:HP], 1.0)

                    # U for this chunk -> accumulate local prefix in SBUF
                    for hp in range(NHP):
                        psU = ps1.tile([128, HP], F32, name="pstr", bufs=4,
                                       padded_shape=[128, 512])
                        for sub in range(2):
                            h = 2 * hp + sub
                            nc.tensor.matmul(
                                psU[sub * 64:(sub + 1) * 64, :],
                                sk_sb[st].bitcast(F32)[:, h * DH:(h + 1) * DH],
                                v_sb[st].bitcast(F32)[:, h * HP:(h + 1) * HP],
                                start=True, stop=True,
                                tile_position=(0, 64 * sub))
                        dest = L_sb[st + 1] if st < NCH - 1 else ball
                        dsl = slice(hp * HP, (hp + 1) * HP)
                        if st == 0:
                            nc.vector.tensor_copy(dest[:, dsl], psU[:])
                        else:
                            nc.vector.tensor_add(dest[:, dsl], psU[:],
                                                 L_sb[st][:, dsl])

                # ---- collective: allgather block sums (trigger early) -----
                cc_in = drpool.tile([128, W], F32)
                cc_out = drpool.tile([N_CORES, 128, W], F32,
                                     addr_space="Shared")
                nc.sync.dma_start(cc_in[:], ball[:])
                nc.gpsimd.collective_compute(
                    "AllGather", Alu.bypass,
                    replica_groups=[list(range(N_CORES))],
                    ins=[cc_in[:]], outs=[cc_out[:]])

                # ---- qT / kT projections + elu (overlap collective) -------
                sqT = [dpool.tile([128, S_BLK], F32R, name=f"sqT{hp}")
                       for hp in range(NHP)]
                skT = [dpool.tile([128, S_BLK], F32R, name=f"skT{hp}")
                       for hp in range(NHP)]
                for hp in range(NHP):
                    for (wt, bias, dst) in ((wq_t, bqt, sqT), (wk_t, bkt, skT)):
                        psq = ps1.tile([128, S_BLK], F32, name="psbig", bufs=3)
                        for dt in range(4):
                            nc.tensor.matmul(
                                psq[:],
                                wt[dt][:, hp * 128:(hp + 1) * 128],
                                hsT[dt][:], start=(dt == 0), stop=(dt == 3))
                        e_t = tpool.tile([128, S_BLK], F32, name="elu_e")
                        r_t = tpool.tile([128, S_BLK], F32, name="elu_r")
                        nc.scalar.activation(e_t[:], psq[:], Act.Exp,
                                             bias=bias[:, hp:hp + 1])
                        nc.scalar.activation(r_t[:], psq[:], Act.Relu,
                                             bias=bias[:, hp:hp + 1])
                        nc.vector.scalar_tensor_tensor(
                            dst[hp][:], e_t[:], 1.0, r_t[:],
                            op0=Alu.min, op1=Alu.add)

                # ---- pre-P pass: scores + causal mask for all chunks ------
                ni_sb = [[None] * NHP for _ in range(NCH)]
                with tc.tile_pool(name="am", bufs=1) as ampool:
                    # local-state seed G0_c = mz + L_c (P-independent)
                    G0 = [mz]
                    for c in range(1, NCH):
                        g0 = dpool.tile([128, W], F32R, name=f"G0_{c}")
                        nc.vector.tensor_add(g0[:], mz[:], L_sb[c][:])
                        G0.append(g0)
                    for c in range(NCH):
                        cs = slice(c * 128, (c + 1) * 128)
                        for hp in range(NHP):
                            am = ampool.tile([128, 256], F32R,
                                             name=f"am{c}_{hp}", bufs=2,
                                             tag="am")
                            ni = ampool.tile([128, 2 * HP], F32,
                                             name=f"ni{c}_{hp}")
                            for sub in range(2):
                                h = 2 * hp + sub
                                hb = slice(sub * 64, (sub + 1) * 64)
                                asl = slice(sub * 128, (sub + 1) * 128)
                                psA = ps1.tile([128, 128], F32, name="pstr",
                                               bufs=4)
                                nc.tensor.matmul(
                                    psA[:], skT[hp][hb, cs], sqT[hp][hb, cs],
                                    start=True, stop=True)
                                nc.vector.tensor_mul(
                                    am[:, asl], psA[:], triu2[:, 0:128])
                                # intra-chunk numerator (P-independent)
                                psNi = ps1.tile([128, HP], F32, name="pstr",
                                                bufs=4)
                                nc.tensor.matmul(
                                    psNi[:], am[:, asl],
                                    v_sb[c][:, h * HP:(h + 1) * HP],
                                    start=True, stop=False)
                                nc.tensor.matmul(
                                    psNi[:], sqT[hp][hb, cs],
                                    G0[c][hb, hp * HP:(hp + 1) * HP],
                                    start=False, stop=True)
                                nc.vector.tensor_copy(
                                    ni[:, sub * HP:(sub + 1) * HP], psNi[:])
                            ni_sb[c][hp] = ni

                    # ---- prefix state P = mz + sum_{j<me} Bg_j ------------
                    bg = dpool.tile([128, N_CORES * W], F32, name="bg")
                    nc.sync.dma_start(
                        bg.rearrange("p (j e) -> p j e", j=N_CORES),
                        cc_out.rearrange("j p e -> p j e"))
                    pa = dpool.tile([128, W], F32R, name="pa")
                    pb = dpool.tile([128, W], F32R, name="pb")
                    nc.vector.tensor_scalar_mul(pa[:], bg[:, 0:W],
                                                pmask[:, 0:1])
                    acc_src = pa
                    for j in range(1, N_CORES - 1):
                        acc_dst = pb if j % 2 == 1 else pa
                        nc.vector.scalar_tensor_tensor(
                            acc_dst[:], bg[:, j * W:(j + 1) * W],
                            pmask[:, j:j + 1], acc_src[:],
                            op0=Alu.mult, op1=Alu.add)
                        acc_src = acc_dst
                    PP = acc_src

                    _ctx["ni_sb"] = ni_sb
                    _ctx["PP"] = PP

            ni_sb = _ctx["ni_sb"]
            PP = _ctx["PP"]
            # ---- post-P pass: inter term, divide, transpose ---------------
            attnT = [dpool.tile([128, S_BLK], F32R, name=f"attnT{hp}")
                     for hp in range(NHP)]
            with tc.tile_pool(name="ps2", bufs=1, space="PSUM") as ps2:
                for c in range(NCH):
                    cs = slice(c * 128, (c + 1) * 128)
                    for hp in range(NHP):
                        ap_ = tpool.tile([128, 128], F32, name="attnp")
                        for sub in range(2):
                            hb = slice(sub * 64, (sub + 1) * 64)
                            nsl = slice(sub * HP, (sub + 1) * HP)
                            psN = ps2.tile([128, HP], F32, name="psN", bufs=3)
                            nc.tensor.matmul(
                                psN[:], sqT[hp][hb, cs],
                                PP[hb, hp * HP:(hp + 1) * HP],
                                start=True, stop=True)
                            num = tpool.tile([128, HP], F32, name="numf",
                                             bufs=3)
                            nc.vector.tensor_add(num[:], psN[:],
                                                 ni_sb[c][hp][:, nsl])
                            den = spool.tile([128, 1], F32, name="den")
                            nc.vector.tensor_scalar_add(den[:], num[:, DH:DH + 1],
                                                        EPS)
                            rec = spool.tile([128, 1], F32, name="rec")
                            nc.vector.reciprocal(rec[:], den[:])
                            nc.vector.tensor_scalar_mul(
                                ap_[:, sub * DH:(sub + 1) * DH],
                                num[:, 0:DH], rec[:])
                        psT = ps2.tile([128, 128], F32, name="pstr2", bufs=2)
                        nc.tensor.transpose(psT[:], ap_[:], ident[:])
                        nc.vector.tensor_copy(attnT[hp][:, cs], psT[:])

                # ---- output projection ------------------------------------
                for st in range(4):
                    ss = slice(st * 128, (st + 1) * 128)
                    psO = ps2.tile([128, D], F32, name="psO", bufs=2)
                    for hp in range(NHP):
                        nc.tensor.matmul(
                            psO[:], attnT[hp][:, ss], wo_t[hp][:],
                            start=(hp == 0), stop=(hp == NHP - 1))
                    y_sb = tpool.tile([128, D], F32, name="ysb", bufs=2)
                    nc.vector.tensor_copy(y_sb[:], psO[:])
                    nc.sync.dma_start(y_d[ss, :], y_sb[:])

    nc.compile()
    return nc


def _get_nc():
    if "nc" not in _CACHE:
        _CACHE["nc"] = _build()
    return _CACHE["nc"]


def _make_in_maps(hidden_states, Wq, bq, Wk, bk, Wv, bv, Wo, M_mem, z_mem):
    hs = np.asarray(hidden_states, np.float32).reshape(S, D)
    Wq = np.ascontiguousarray(np.asarray(Wq, np.float32))
    Wk = np.ascontiguousarray(np.asarray(Wk, np.float32))
    Wv = np.ascontiguousarray(np.asarray(Wv, np.float32))
    Wo = np.ascontiguousarray(np.asarray(Wo, np.float32))
    bq = np.asarray(bq, np.float32)
    bk = np.asarray(bk, np.float32)
    bv = np.asarray(bv, np.float32)
    M_mem = np.asarray(M_mem, np.float32)
    z_mem = np.asarray(z_mem, np.float32)

    bqt = np.ascontiguousarray(bq.reshape(NHP, 128).T)
    bkt = np.ascontiguousarray(bk.reshape(NHP, 128).T)

    mz = np.zeros((128, W), np.float32)
    for h in range(H):
        pr, col = (h % 2) * 64, (h // 2) * HP
        mz[pr:pr + 64, col:col + DH] = M_mem[h]
        mz[pr:pr + 64, col + DH] = z_mem[h]

    in_maps = []
    for c in range(N_CORES):
        pm = np.zeros((128, N_CORES), np.float32)
        pm[:, :c] = 1.0
        in_maps.append({
            "hs": np.ascontiguousarray(hs[c * S_BLK:(c + 1) * S_BLK]),
            "wq": Wq, "wk": Wk, "wv": Wv, "wo": Wo,
            "bqt": bqt, "bkt": bkt,
            "bkr": bk.reshape(1, D), "bvr": bv.reshape(1, D),
            "mz": mz, "pmask": pm,
        })
    return in_maps


def kernel(hidden_states, Wq, bq, Wk, bk, Wv, bv, Wo, M_mem, z_mem):
    nc = _get_nc()
    in_maps = _make_in_maps(hidden_states, Wq, bq, Wk, bk, Wv, bv, Wo,
                            M_mem, z_mem)
    res = run_bass_kernel_spmd(nc, in_maps, list(range(N_CORES)))
    out = np.concatenate([res.results[c]["y"] for c in range(N_CORES)], axis=0)
    return out.reshape(1, S, D)



# revision 37
# speedup vs baseline: 1.0401x; 1.0401x over previous
"""Trainium2 Bass kernel for LinearMemoryAttention (B=1, S=4096, D=512, H=8, Dh=64).

v3: sequence-parallel over 8 cores (512 tokens each), all heads local.
- bf16 matmul operands throughout (fp32 PSUM accumulation).
- Projections computed feature-major so biases fuse into activations.
- Cross-core causal state exchanged through shared-HBM scratchpad: each
  core scatters its block-sum into its rank's slot (indirect DMA, slot
  index supplied as a per-core input), announces completion with a
  remote semaphore broadcast (SWDGE, no ncfw collective), then gathers
  all 8 slots with one DMA. A 1-byte prelude kernel barrier provides
  entry sync across invocations.

Self-contained: hardcodes all shapes; builds/compiles the Bass program once.
"""

import os

import numpy as np

import concourse.bass as bass
import concourse.bacc as bacc
import concourse.mybir as mybir
import concourse.tile as tile
from concourse.bass_utils import run_bass_kernel_spmd

F32 = mybir.dt.float32
BF16 = mybir.dt.bfloat16
U32 = mybir.dt.uint32
N_CORES = 8
S = 4096
D = 512
H = 8
DH = 64
HP = 66  # head width incl. denominator column (+1 pad)
S_BLK = S // N_CORES  # 512 rows per core
NCH = S_BLK // 128  # 4 chunks of 128
NHP = H // 2  # 4 head pairs
EPS = 1e-6
W = NHP * HP  # 264
AUXW = 4 + 4 + 4 + W + N_CORES  # bqt | bkt | bvt | mz | pmask

_CACHE = {}
DEBUG = os.environ.get("LMA_DEBUG", "")  # "" or "noremote"


def _build():
    Alu = mybir.AluOpType
    Act = mybir.ActivationFunctionType
    nc = bacc.Bacc("TRN2", target_bir_lowering=False, debug=False,
                   num_devices=N_CORES)

    hs_d = nc.dram_tensor("hs", [S_BLK, D], F32, kind="ExternalInput").ap()
    wq_d = nc.dram_tensor("wq", [D, D], F32, kind="ExternalInput").ap()
    wk_d = nc.dram_tensor("wk", [D, D], F32, kind="ExternalInput").ap()
    wv_d = nc.dram_tensor("wv", [D, D], F32, kind="ExternalInput").ap()
    wo_d = nc.dram_tensor("wo", [D, D], F32, kind="ExternalInput").ap()
    aux_d = nc.dram_tensor("aux", [128, AUXW], F32, kind="ExternalInput").ap()
    sidx_d = nc.dram_tensor("sidx", [128, 1], U32, kind="ExternalInput").ap()
    y_d = nc.dram_tensor("y", [S_BLK, D], F32, kind="ExternalOutput").ap()

    rsem = nc.alloc_semaphore("lma_rsem")
    lsem = nc.alloc_semaphore("lma_lsem")
    dsem = nc.alloc_semaphore("lma_dsem")

    with tile.TileContext(nc) as tc:
        with (
            tc.tile_pool(name="const", bufs=1) as cpool,
            tc.tile_pool(name="wstage", bufs=1) as wspool,
            tc.tile_pool(name="wpool", bufs=1) as wpool,
            tc.tile_pool(name="data", bufs=1) as dpool,
            tc.tile_pool(name="tmp", bufs=3) as tpool,
            tc.tile_pool(name="small", bufs=4) as spool,
            tc.tile_pool(name="dram", bufs=1, space="DRAM") as drpool,
        ):
            # ---- input DMAs (one issue per tensor, sync queue = idle) ------
            hs_t = dpool.tile([128, NCH * D], F32, name="hsall")
            nc.sync.dma_start(
                hs_t.rearrange("p (c d) -> p c d", c=NCH),
                hs_d.rearrange("(c p) d -> p c d", p=128))
            aux = cpool.tile([128, AUXW], F32)
            nc.sync.dma_start(aux[:], aux_d[:])
            sidx = cpool.tile([128, 1], U32)
            nc.sync.dma_start(sidx[:], sidx_d[:])
            wk_s = wspool.tile([128, 4 * D], F32, name="wks")
            nc.sync.dma_start(
                wk_s.rearrange("p (c d) -> p c d", c=4),
                wk_d.rearrange("(c p) d -> p c d", p=128))
            wv_s = wspool.tile([128, 4 * D], F32, name="wvs")
            nc.sync.dma_start(
                wv_s.rearrange("p (c d) -> p c d", c=4),
                wv_d.rearrange("(c p) d -> p c d", p=128))
            wq_s = wspool.tile([128, 4 * D], F32, name="wqs")
            nc.sync.dma_start(
                wq_s.rearrange("p (c d) -> p c d", c=4),
                wq_d.rearrange("(c p) d -> p c d", p=128))
            wo_s = wspool.tile([128, 4 * D], F32, name="wos")
            nc.sync.dma_start(
                wo_s.rearrange("p (c d) -> p c d", c=4),
                wo_d.rearrange("(c p) d -> p c d", p=128))

            bqt = aux[:, 0:4]
            bkt = aux[:, 4:8]
            bvt = aux[:, 8:12]
            mz = aux[:, 12:12 + W]
            pmask = aux[:, 12 + W:12 + W + N_CORES]

            # ---- constants -------------------------------------------------
            ones128 = cpool.tile([128, 128], BF16)
            nc.gpsimd.memset(ones128[:], 1.0)
            ident = cpool.tile([128, 128], BF16)
            nc.gpsimd.affine_select(ident[:], ones128[:], pattern=[[1, 128]],
                                    compare_op=Alu.is_equal, fill=0.0, base=0,
                                    channel_multiplier=-1)
            triu = cpool.tile([128, 128], BF16)
            nc.gpsimd.affine_select(triu[:], ones128[:], pattern=[[1, 128]],
                                    compare_op=Alu.is_ge, fill=0.0, base=0,
                                    channel_multiplier=-1)

            # ---- cast hs + weights to bf16 --------------------------------
            hs_b = dpool.tile([128, NCH * D], BF16, name="hsb")
            for c in range(NCH):
                nc.vector.tensor_copy(hs_b[:, c * D:(c + 1) * D],
                                      hs_t[:, c * D:(c + 1) * D])
            wk_t = [wpool.tile([128, D], BF16, name=f"wk{i}") for i in range(4)]
            wv_t = [wpool.tile([128, D], BF16, name=f"wv{i}") for i in range(4)]
            wq_t = [wpool.tile([128, D], BF16, name=f"wq{i}") for i in range(4)]
            wo_t = [wpool.tile([128, D], BF16, name=f"wo{i}") for i in range(4)]
            for i in range(4):
                sl = slice(i * D, (i + 1) * D)
                nc.vector.tensor_copy(wk_t[i][:], wk_s[:, sl])
                nc.vector.tensor_copy(wv_t[i][:], wv_s[:, sl])
                nc.scalar.copy(wq_t[i][:], wq_s[:, sl])
                nc.gpsimd.tensor_copy(wo_t[i][:], wo_s[:, sl])

            # ---- cross-core exchange buffers ------------------------------
            ball = dpool.tile([128, W], BF16, name="ball")
            slots = dpool.tile([128, N_CORES * W], BF16, name="slots")


            hsT = [dpool.tile([128, S_BLK], BF16, name=f"hsT{i}")
                   for i in range(4)]
            skT = [dpool.tile([128, S_BLK], BF16, name=f"skT{hp}")
                   for hp in range(NHP)]
            sqT = [dpool.tile([128, S_BLK], BF16, name=f"sqT{hp}")
                   for hp in range(NHP)]
            vT = [dpool.tile([128, S_BLK], BF16, name=f"vT{hp}")
                  for hp in range(NHP)]
            sk_tm = [dpool.tile([128, H * DH], BF16, name=f"sk{c}")
                     for c in range(NCH)]
            v_tm = [dpool.tile([128, H * HP], BF16, name=f"v{c}")
                    for c in range(NCH)]
            for c in range(NCH):
                v3 = v_tm[c].rearrange("p (h e) -> p h e", e=HP)
                nc.gpsimd.memset(v3[:, :, DH:HP], 1.0)
            L_sb = [None] + [dpool.tile([128, W], F32, name=f"L{c}")
                             for c in range(1, NCH)]
            ball_f = dpool.tile([128, W], F32, name="ballf")

            with tc.tile_pool(name="ps", bufs=1, space="PSUM") as ps:
                # ---- hs transposes -----------------------------------------
                for dt in range(4):
                    for st in range(4):
                        pst = ps.tile([128, 128], BF16, name="pstr", bufs=2)
                        nc.tensor.transpose(
                            pst[:],
                            hs_b[:, st * D + dt * 128: st * D + (dt + 1) * 128],
                            ident[:])
                        nc.vector.tensor_copy(
                            hsT[dt][:, st * 128:(st + 1) * 128], pst[:])

                # ---- k, v projections (feature-major), elu(k)+1 ------------
                for hp in range(NHP):
                    fs = slice(hp * 128, (hp + 1) * 128)
                    psk = ps.tile([128, S_BLK], F32, name="psbig", bufs=2)
                    for dt in range(4):
                        nc.tensor.matmul(psk[:], wk_t[dt][:, fs], hsT[dt][:],
                                         start=(dt == 0), stop=(dt == 3))
                    e_t = tpool.tile([128, S_BLK], BF16, name="elu_e")
                    r_t = tpool.tile([128, S_BLK], BF16, name="elu_r")
                    nc.scalar.activation(e_t[:], psk[:], Act.Exp,
                                         bias=bkt[:, hp:hp + 1])
                    nc.vector.tensor_scalar(r_t[:], psk[:], bkt[:, hp:hp + 1],
                                            0.0, op0=Alu.add, op1=Alu.max)
                    nc.vector.scalar_tensor_tensor(
                        skT[hp][:], e_t[:], 1.0, r_t[:],
                        op0=Alu.min, op1=Alu.add)

                    psv = ps.tile([128, S_BLK], F32, name="psbig", bufs=2)
                    for dt in range(4):
                        nc.tensor.matmul(psv[:], wv_t[dt][:, fs], hsT[dt][:],
                                         start=(dt == 0), stop=(dt == 3))
                    nc.scalar.activation(vT[hp][:], psv[:], Act.Identity,
                                         bias=bvt[:, hp:hp + 1])

                # ---- transpose sk, v to token-major ------------------------
                for c in range(NCH):
                    cs = slice(c * 128, (c + 1) * 128)
                    for hp in range(NHP):
                        pst = ps.tile([128, 128], BF16, name="pstr", bufs=2)
                        nc.tensor.transpose(pst[:], skT[hp][:, cs], ident[:])
                        nc.vector.tensor_copy(
                            sk_tm[c][:, hp * 128:(hp + 1) * 128], pst[:])
                        pst2 = ps.tile([128, 128], BF16, name="pstr", bufs=2)
                        nc.tensor.transpose(pst2[:], vT[hp][:, cs], ident[:])
                        v3 = v_tm[c].rearrange("p (h e) -> p h e", e=HP)
                        nc.vector.tensor_copy(
                            v3[:, 2 * hp:2 * hp + 2, 0:DH],
                            pst2.rearrange("p (h e) -> p h e", e=DH))

                # ---- U outer products -> local prefix + block total --------
                for st in range(NCH):
                    for hp in range(NHP):
                        psU = ps.tile([128, HP], F32, name="psu", bufs=2,
                                      padded_shape=[128, 512])
                        for sub in range(2):
                            h = 2 * hp + sub
                            nc.tensor.matmul(
                                psU[sub * 64:(sub + 1) * 64, :],
                                sk_tm[st][:, h * DH:(h + 1) * DH],
                                v_tm[st][:, h * HP:(h + 1) * HP],
                                start=True, stop=True,
                                tile_position=(0, 64 * sub))
                        dest = L_sb[st + 1] if st < NCH - 1 else ball_f
                        dsl = slice(hp * HP, (hp + 1) * HP)
                        if st == 0:
                            nc.vector.tensor_copy(dest[:, dsl], psU[:])
                        else:
                            nc.vector.tensor_add(dest[:, dsl], psU[:],
                                                 L_sb[st][:, dsl])

                nc.gpsimd.tensor_copy(ball[:], ball_f[:])

                # ---- exchange: AllGather block totals (bf16 payload) -------
                if DEBUG != "noremote":
                    cc_in = drpool.tile([128, W], BF16, name="ccin")
                    cc_out = drpool.tile([N_CORES, 128, W], BF16,
                                         addr_space="Shared", name="ccout")
                    nc.sync.dma_start(cc_in[:], ball[:])
                    nc.gpsimd.collective_compute(
                        "AllGather", Alu.bypass,
                        replica_groups=[list(range(N_CORES))],
                        ins=[cc_in[:]], outs=[cc_out[:]])

                # ---- q projections (overlap the exchange) ------------------
                for hp in range(NHP):
                    fs = slice(hp * 128, (hp + 1) * 128)
                    psq = ps.tile([128, S_BLK], F32, name="psbig", bufs=2)
                    for dt in range(4):
                        nc.tensor.matmul(psq[:], wq_t[dt][:, fs], hsT[dt][:],
                                         start=(dt == 0), stop=(dt == 3))
                    e_t = tpool.tile([128, S_BLK], BF16, name="elu_e")
                    r_t = tpool.tile([128, S_BLK], BF16, name="elu_r")
                    nc.scalar.activation(e_t[:], psq[:], Act.Exp,
                                         bias=bqt[:, hp:hp + 1])
                    nc.vector.tensor_scalar(r_t[:], psq[:], bqt[:, hp:hp + 1],
                                            0.0, op0=Alu.add, op1=Alu.max)
                    nc.vector.scalar_tensor_tensor(
                        sqT[hp][:], e_t[:], 1.0, r_t[:],
                        op0=Alu.min, op1=Alu.add)

                # ---- masked intra-chunk scores -----------------------------
                am_sb = [[None] * NHP for _ in range(NCH)]
                for c in range(NCH):
                    cs = slice(c * 128, (c + 1) * 128)
                    for hp in range(NHP):
                        am = dpool.tile([128, 256], BF16, name=f"am{c}_{hp}")
                        for sub in range(2):
                            hb = slice(sub * 64, (sub + 1) * 64)
                            psA = ps.tile([128, 128], F32, name="psa", bufs=2)
                            nc.tensor.matmul(psA[:], skT[hp][hb, cs],
                                             sqT[hp][hb, cs],
                                             start=True, stop=True)
                            nc.vector.tensor_mul(
                                am[:, sub * 128:(sub + 1) * 128],
                                psA[:], triu[:])
                        am_sb[c][hp] = am

                # ---- gather slots, combine prefix state P ------------------
                if DEBUG != "noremote":
                    nc.sync.dma_start(
                        slots.rearrange("p (j e) -> p j e", j=N_CORES),
                        cc_out.rearrange("j p e -> p j e"))

                PM = dpool.tile([128, W], F32, name="PM")
                PPc = [dpool.tile([128, W], BF16, name=f"PPc{c}")
                       for c in range(NCH)]
                if DEBUG == "noremote" or os.environ.get("LMA_NOGATHER") == "1":
                    nc.vector.tensor_copy(PM[:], mz[:])
                else:
                    nc.vector.scalar_tensor_tensor(
                        PM[:], slots[:, 0:W], pmask[:, 0:1], mz[:],
                        op0=Alu.mult, op1=Alu.add)
                    for k in range(1, N_CORES):
                        nc.vector.scalar_tensor_tensor(
                            PM[:], slots[:, k * W:(k + 1) * W],
                            pmask[:, k:k + 1], PM[:],
                            op0=Alu.mult, op1=Alu.add)
                nc.vector.tensor_copy(PPc[0][:], PM[:])
                for c in range(1, NCH):
                    nc.vector.tensor_add(PPc[c][:], PM[:], L_sb[c][:])

                # ---- numerators, divide, transpose -------------------------
                attnT = [dpool.tile([128, S_BLK], BF16, name=f"attnT{hp}")
                         for hp in range(NHP)]
                for c in range(NCH):
                    cs = slice(c * 128, (c + 1) * 128)
                    for hp in range(NHP):
                        ap_ = tpool.tile([128, 128], BF16, name="attnp")
                        for sub in range(2):
                            h = 2 * hp + sub
                            hb = slice(sub * 64, (sub + 1) * 64)
                            psN = ps.tile([128, HP], F32, name="psu", bufs=2,
                                          padded_shape=[128, 512])
                            nc.tensor.matmul(
                                psN[:],
                                am_sb[c][hp][:, sub * 128:(sub + 1) * 128],
                                v_tm[c][:, h * HP:(h + 1) * HP],
                                start=True, stop=False)
                            nc.tensor.matmul(
                                psN[:], sqT[hp][hb, cs],
                                PPc[c][hb, hp * HP:(hp + 1) * HP],
                                start=False, stop=True)
                            den = spool.tile([128, 1], F32, name="den")
                            nc.vector.tensor_scalar_add(
                                den[:], psN[:, DH:DH + 1], EPS)
                            rec = spool.tile([128, 1], F32, name="rec")
                            nc.vector.reciprocal(rec[:], den[:])
                            nc.vector.tensor_scalar_mul(
                                ap_[:, sub * DH:(sub + 1) * DH],
                                psN[:, 0:DH], rec[:])
                        psT = ps.tile([128, 128], BF16, name="pstr", bufs=2)
                        nc.tensor.transpose(psT[:], ap_[:], ident[:])
                        nc.vector.tensor_copy(attnT[hp][:, cs], psT[:])

                # ---- output projection -------------------------------------
                for st in range(NCH):
                    ss = slice(st * 128, (st + 1) * 128)
                    psO = ps.tile([128, D], F32, name="psbig", bufs=2)
                    for hp in range(NHP):
                        nc.tensor.matmul(psO[:], attnT[hp][:, ss], wo_t[hp][:],
                                         start=(hp == 0), stop=(hp == NHP - 1))
                    y_sb = tpool.tile([128, D], F32, name="ysb", bufs=2)
                    nc.vector.tensor_copy(y_sb[:], psO[:])
                    nc.sync.dma_start(y_d[ss, :], y_sb[:])

    nc.compile()
    return nc


def _get_nc():
    if "nc" not in _CACHE:
        _CACHE["nc"] = _build()
    return _CACHE["nc"]


def _make_in_maps(hidden_states, Wq, bq, Wk, bk, Wv, bv, Wo, M_mem, z_mem):
    hs = np.asarray(hidden_states, np.float32).reshape(S, D)
    Wq = np.ascontiguousarray(np.asarray(Wq, np.float32))
    Wk = np.ascontiguousarray(np.asarray(Wk, np.float32))
    Wv = np.ascontiguousarray(np.asarray(Wv, np.float32))
    Wo = np.ascontiguousarray(np.asarray(Wo, np.float32))
    bq = np.asarray(bq, np.float32)
    bk = np.asarray(bk, np.float32)
    bv = np.asarray(bv, np.float32)
    M_mem = np.asarray(M_mem, np.float32)
    z_mem = np.asarray(z_mem, np.float32)

    mz = np.zeros((128, W), np.float32)
    for h in range(H):
        pr, col = (h % 2) * 64, (h // 2) * HP
        mz[pr:pr + 64, col:col + DH] = M_mem[h]
        mz[pr:pr + 64, col + DH] = z_mem[h]

    in_maps = []
    for c in range(N_CORES):
        aux = np.zeros((128, AUXW), np.float32)
        aux[:, 0:4] = bq.reshape(NHP, 128).T
        aux[:, 4:8] = bk.reshape(NHP, 128).T
        aux[:, 8:12] = bv.reshape(NHP, 128).T
        aux[:, 12:12 + W] = mz
        aux[:, 12 + W:12 + W + c] = 1.0
        sidx = np.full(128, c, dtype=np.uint32)
        in_maps.append({
            "hs": np.ascontiguousarray(hs[c * S_BLK:(c + 1) * S_BLK]),
            "wq": Wq, "wk": Wk, "wv": Wv, "wo": Wo,
            "aux": aux,
            "sidx": sidx.reshape(128, 1),
        })
    return in_maps


def kernel(hidden_states, Wq, bq, Wk, bk, Wv, bv, Wo, M_mem, z_mem):
    nc = _get_nc()
    in_maps = _make_in_maps(hidden_states, Wq, bq, Wk, bk, Wv, bv, Wo,
                            M_mem, z_mem)
    res = run_bass_kernel_spmd(nc, in_maps, list(range(N_CORES)))
    out = np.concatenate([res.results[c]["y"] for c in range(N_CORES)], axis=0)
    return out.reshape(1, S, D)


# revision 39
# speedup vs baseline: 1.1064x; 1.0637x over previous
"""Trainium2 Bass kernel for LinearMemoryAttention (B=1, S=4096, D=512, H=8, Dh=64).

v3: sequence-parallel over 8 cores (512 tokens each), all heads local.
- bf16 matmul operands throughout (fp32 PSUM accumulation).
- Projections computed feature-major so biases fuse into activations.
- Cross-core causal state exchanged through shared-HBM scratchpad: each
  core scatters its block-sum into its rank's slot (indirect DMA, slot
  index supplied as a per-core input), announces completion with a
  remote semaphore broadcast (SWDGE, no ncfw collective), then gathers
  all 8 slots with one DMA. A 1-byte prelude kernel barrier provides
  entry sync across invocations.

Self-contained: hardcodes all shapes; builds/compiles the Bass program once.
"""

import os

import numpy as np

import concourse.bass as bass
import concourse.bacc as bacc
import concourse.mybir as mybir
import concourse.tile as tile
from concourse.bass_utils import run_bass_kernel_spmd

F32 = mybir.dt.float32
BF16 = mybir.dt.bfloat16
U32 = mybir.dt.uint32
N_CORES = 8
S = 4096
D = 512
H = 8
DH = 64
HP = 66  # head width incl. denominator column (+1 pad)
S_BLK = S // N_CORES  # 512 rows per core
NCH = S_BLK // 128  # 4 chunks of 128
NHP = H // 2  # 4 head pairs
EPS = 1e-6
W = NHP * HP  # 264
AUXW = 4 + 4 + 4 + W + N_CORES  # bqt | bkt | bvt | mz | pmask

_CACHE = {}
DEBUG = os.environ.get("LMA_DEBUG", "")  # "" or "noremote"


def _build():
    Alu = mybir.AluOpType
    Act = mybir.ActivationFunctionType
    nc = bacc.Bacc("TRN2", target_bir_lowering=False, debug=False,
                   num_devices=N_CORES)

    hs_d = nc.dram_tensor("hs", [S_BLK, D], F32, kind="ExternalInput").ap()
    wq_d = nc.dram_tensor("wq", [D, D], F32, kind="ExternalInput").ap()
    wk_d = nc.dram_tensor("wk", [D, D], F32, kind="ExternalInput").ap()
    wv_d = nc.dram_tensor("wv", [D, D], F32, kind="ExternalInput").ap()
    wo_d = nc.dram_tensor("wo", [D, D], F32, kind="ExternalInput").ap()
    aux_d = nc.dram_tensor("aux", [128, AUXW], F32, kind="ExternalInput").ap()
    sidx_d = nc.dram_tensor("sidx", [128, 1], U32, kind="ExternalInput").ap()
    y_d = nc.dram_tensor("y", [S_BLK, D], F32, kind="ExternalOutput").ap()

    rsem = nc.alloc_semaphore("lma_rsem")
    lsem = nc.alloc_semaphore("lma_lsem")
    dsem = nc.alloc_semaphore("lma_dsem")

    with tile.TileContext(nc) as tc:
        with (
            tc.tile_pool(name="const", bufs=1) as cpool,
            tc.tile_pool(name="wstage", bufs=1) as wspool,
            tc.tile_pool(name="wpool", bufs=1) as wpool,
            tc.tile_pool(name="data", bufs=1) as dpool,
            tc.tile_pool(name="tmp", bufs=3) as tpool,
            tc.tile_pool(name="small", bufs=4) as spool,
            tc.tile_pool(name="dram", bufs=1, space="DRAM") as drpool,
        ):
            # ---- input DMAs (one issue per tensor, sync queue = idle) ------
            hs_t = dpool.tile([128, NCH * D], F32, name="hsall")
            nc.sync.dma_start(
                hs_t.rearrange("p (c d) -> p c d", c=NCH),
                hs_d.rearrange("(c p) d -> p c d", p=128))
            aux = cpool.tile([128, AUXW], F32)
            nc.sync.dma_start(aux[:], aux_d[:])
            sidx = cpool.tile([128, 1], U32)
            nc.sync.dma_start(sidx[:], sidx_d[:])
            wk_s = wspool.tile([128, 4 * D], F32, name="wks")
            nc.sync.dma_start(
                wk_s.rearrange("p (c d) -> p c d", c=4),
                wk_d.rearrange("(c p) d -> p c d", p=128))
            wv_s = wspool.tile([128, 4 * D], F32, name="wvs")
            nc.sync.dma_start(
                wv_s.rearrange("p (c d) -> p c d", c=4),
                wv_d.rearrange("(c p) d -> p c d", p=128))
            wq_s = wspool.tile([128, 4 * D], F32, name="wqs")
            nc.sync.dma_start(
                wq_s.rearrange("p (c d) -> p c d", c=4),
                wq_d.rearrange("(c p) d -> p c d", p=128))
            wo_s = wspool.tile([128, 4 * D], F32, name="wos")
            nc.sync.dma_start(
                wo_s.rearrange("p (c d) -> p c d", c=4),
                wo_d.rearrange("(c p) d -> p c d", p=128))

            bqt = aux[:, 0:4]
            bkt = aux[:, 4:8]
            bvt = aux[:, 8:12]
            mz = aux[:, 12:12 + W]
            pmask = aux[:, 12 + W:12 + W + N_CORES]

            # ---- constants -------------------------------------------------
            ones128 = cpool.tile([128, 128], BF16)
            nc.gpsimd.memset(ones128[:], 1.0)
            ident = cpool.tile([128, 128], BF16)
            nc.gpsimd.affine_select(ident[:], ones128[:], pattern=[[1, 128]],
                                    compare_op=Alu.is_equal, fill=0.0, base=0,
                                    channel_multiplier=-1)
            triu = cpool.tile([128, 128], BF16)
            nc.gpsimd.affine_select(triu[:], ones128[:], pattern=[[1, 128]],
                                    compare_op=Alu.is_ge, fill=0.0, base=0,
                                    channel_multiplier=-1)

            # ---- cast hs + weights to bf16 --------------------------------
            hs_b = dpool.tile([128, NCH * D], BF16, name="hsb")
            for c in range(NCH):
                nc.vector.tensor_copy(hs_b[:, c * D:(c + 1) * D],
                                      hs_t[:, c * D:(c + 1) * D])
            wk_t = [wpool.tile([128, D], BF16, name=f"wk{i}") for i in range(4)]
            wv_t = [wpool.tile([128, D], BF16, name=f"wv{i}") for i in range(4)]
            wq_t = [wpool.tile([128, D], BF16, name=f"wq{i}") for i in range(4)]
            wo_t = [wpool.tile([128, D], BF16, name=f"wo{i}") for i in range(4)]
            for i in range(4):
                sl = slice(i * D, (i + 1) * D)
                nc.vector.tensor_copy(wk_t[i][:], wk_s[:, sl])
                nc.vector.tensor_copy(wv_t[i][:], wv_s[:, sl])
                nc.scalar.copy(wq_t[i][:], wq_s[:, sl])
                nc.scalar.copy(wo_t[i][:], wo_s[:, sl])

            # ---- cross-core exchange buffers ------------------------------
            ball = dpool.tile([128, W], BF16, name="ball")
            slots = dpool.tile([128, N_CORES * W], BF16, name="slots")


            hsT = [dpool.tile([128, S_BLK], BF16, name=f"hsT{i}")
                   for i in range(4)]
            skT = [dpool.tile([128, S_BLK], BF16, name=f"skT{hp}")
                   for hp in range(NHP)]
            sqT = [dpool.tile([128, S_BLK], BF16, name=f"sqT{hp}")
                   for hp in range(NHP)]
            vT = [dpool.tile([128, S_BLK], BF16, name=f"vT{hp}")
                  for hp in range(NHP)]
            sk_tm = [dpool.tile([128, H * DH], BF16, name=f"sk{c}")
                     for c in range(NCH)]
            v_tm = [dpool.tile([128, H * HP], BF16, name=f"v{c}")
                    for c in range(NCH)]
            for c in range(NCH):
                v3 = v_tm[c].rearrange("p (h e) -> p h e", e=HP)
                nc.gpsimd.memset(v3[:, :, DH:HP], 1.0)
            L_sb = [None] + [dpool.tile([128, W], F32, name=f"L{c}")
                             for c in range(1, NCH)]
            ball_f = dpool.tile([128, W], F32, name="ballf")

            with tc.tile_pool(name="ps", bufs=1, space="PSUM") as ps:
                # ---- hs transposes -----------------------------------------
                for dt in range(4):
                    for st in range(4):
                        pst = ps.tile([128, 128], BF16, name="pstr", bufs=2)
                        nc.tensor.transpose(
                            pst[:],
                            hs_b[:, st * D + dt * 128: st * D + (dt + 1) * 128],
                            ident[:])
                        nc.vector.tensor_copy(
                            hsT[dt][:, st * 128:(st + 1) * 128], pst[:])

                # ---- k, v projections (feature-major), elu(k)+1 ------------
                for hp in range(NHP):
                    fs = slice(hp * 128, (hp + 1) * 128)
                    psk = ps.tile([128, S_BLK], F32, name="psbig", bufs=2)
                    for dt in range(4):
                        nc.tensor.matmul(psk[:], wk_t[dt][:, fs], hsT[dt][:],
                                         start=(dt == 0), stop=(dt == 3))
                    e_t = tpool.tile([128, S_BLK], BF16, name="elu_e")
                    r_t = tpool.tile([128, S_BLK], BF16, name="elu_r")
                    nc.scalar.activation(e_t[:], psk[:], Act.Exp,
                                         bias=bkt[:, hp:hp + 1])
                    nc.vector.tensor_scalar(r_t[:], psk[:], bkt[:, hp:hp + 1],
                                            0.0, op0=Alu.add, op1=Alu.max)
                    nc.vector.scalar_tensor_tensor(
                        skT[hp][:], e_t[:], 1.0, r_t[:],
                        op0=Alu.min, op1=Alu.add)

                    psv = ps.tile([128, S_BLK], F32, name="psbig", bufs=2)
                    for dt in range(4):
                        nc.tensor.matmul(psv[:], wv_t[dt][:, fs], hsT[dt][:],
                                         start=(dt == 0), stop=(dt == 3))
                    nc.scalar.activation(vT[hp][:], psv[:], Act.Identity,
                                         bias=bvt[:, hp:hp + 1])

                # ---- transpose sk, v to token-major ------------------------
                for c in range(NCH):
                    cs = slice(c * 128, (c + 1) * 128)
                    for hp in range(NHP):
                        pst = ps.tile([128, 128], BF16, name="pstr", bufs=2)
                        nc.tensor.transpose(pst[:], skT[hp][:, cs], ident[:])
                        nc.vector.tensor_copy(
                            sk_tm[c][:, hp * 128:(hp + 1) * 128], pst[:])
                        pst2 = ps.tile([128, 128], BF16, name="pstr", bufs=2)
                        nc.tensor.transpose(pst2[:], vT[hp][:, cs], ident[:])
                        v3 = v_tm[c].rearrange("p (h e) -> p h e", e=HP)
                        nc.scalar.copy(
                            v3[:, 2 * hp:2 * hp + 2, 0:DH],
                            pst2.rearrange("p (h e) -> p h e", e=DH))

                # ---- U outer products -> local prefix + block total --------
                for st in range(NCH):
                    for hp in range(NHP):
                        psU = ps.tile([128, HP], F32, name="psu", bufs=2,
                                      padded_shape=[128, 512])
                        for sub in range(2):
                            h = 2 * hp + sub
                            nc.tensor.matmul(
                                psU[sub * 64:(sub + 1) * 64, :],
                                sk_tm[st][:, h * DH:(h + 1) * DH],
                                v_tm[st][:, h * HP:(h + 1) * HP],
                                start=True, stop=True,
                                tile_position=(0, 64 * sub))
                        dest = L_sb[st + 1] if st < NCH - 1 else ball_f
                        dsl = slice(hp * HP, (hp + 1) * HP)
                        nc.scalar.copy(dest[:, dsl], psU[:])

                for st in range(2, NCH):
                    nc.gpsimd.tensor_add(L_sb[st][:], L_sb[st][:],
                                         L_sb[st - 1][:])
                nc.gpsimd.tensor_add(ball_f[:], ball_f[:],
                                     L_sb[NCH - 1][:])
                nc.scalar.copy(ball[:], ball_f[:])

                # ---- exchange: AllGather block totals (bf16 payload) -------
                if DEBUG != "noremote":
                    cc_in = drpool.tile([128, W], BF16, name="ccin")
                    cc_out = drpool.tile([N_CORES, 128, W], BF16,
                                         addr_space="Shared", name="ccout")
                    nc.sync.dma_start(cc_in[:], ball[:])
                    nc.gpsimd.collective_compute(
                        "AllGather", Alu.bypass,
                        replica_groups=[list(range(N_CORES))],
                        ins=[cc_in[:]], outs=[cc_out[:]])

                # ---- q projections (overlap the exchange) ------------------
                for hp in range(NHP):
                    fs = slice(hp * 128, (hp + 1) * 128)
                    psq = ps.tile([128, S_BLK], F32, name="psbig", bufs=2)
                    for dt in range(4):
                        nc.tensor.matmul(psq[:], wq_t[dt][:, fs], hsT[dt][:],
                                         start=(dt == 0), stop=(dt == 3))
                    e_t = tpool.tile([128, S_BLK], BF16, name="elu_e")
                    r_t = tpool.tile([128, S_BLK], BF16, name="elu_r")
                    nc.scalar.activation(e_t[:], psq[:], Act.Exp,
                                         bias=bqt[:, hp:hp + 1])
                    nc.vector.tensor_scalar(r_t[:], psq[:], bqt[:, hp:hp + 1],
                                            0.0, op0=Alu.add, op1=Alu.max)
                    nc.vector.scalar_tensor_tensor(
                        sqT[hp][:], e_t[:], 1.0, r_t[:],
                        op0=Alu.min, op1=Alu.add)

                # ---- masked intra-chunk scores -----------------------------
                am_sb = [[None] * NHP for _ in range(NCH)]
                for c in range(NCH):
                    cs = slice(c * 128, (c + 1) * 128)
                    for hp in range(NHP):
                        am = dpool.tile([128, 256], BF16, name=f"am{c}_{hp}")
                        for sub in range(2):
                            hb = slice(sub * 64, (sub + 1) * 64)
                            psA = ps.tile([128, 128], F32, name="psa", bufs=2)
                            nc.tensor.matmul(psA[:], skT[hp][hb, cs],
                                             sqT[hp][hb, cs],
                                             start=True, stop=True)
                            nc.vector.tensor_mul(
                                am[:, sub * 128:(sub + 1) * 128],
                                psA[:], triu[:])
                        am_sb[c][hp] = am

                # ---- gather slots, combine prefix state P ------------------
                if DEBUG != "noremote":
                    nc.sync.dma_start(
                        slots.rearrange("p (j e) -> p j e", j=N_CORES),
                        cc_out.rearrange("j p e -> p j e"))

                PM = dpool.tile([128, W], F32, name="PM")
                PPc = [dpool.tile([128, W], BF16, name=f"PPc{c}")
                       for c in range(NCH)]
                if DEBUG == "noremote" or os.environ.get("LMA_NOGATHER") == "1":
                    nc.vector.tensor_copy(PM[:], mz[:])
                else:
                    nc.vector.scalar_tensor_tensor(
                        PM[:], slots[:, 0:W], pmask[:, 0:1], mz[:],
                        op0=Alu.mult, op1=Alu.add)
                    for k in range(1, N_CORES):
                        nc.vector.scalar_tensor_tensor(
                            PM[:], slots[:, k * W:(k + 1) * W],
                            pmask[:, k:k + 1], PM[:],
                            op0=Alu.mult, op1=Alu.add)
                nc.vector.tensor_copy(PPc[0][:], PM[:])
                for c in range(1, NCH):
                    nc.vector.tensor_add(PPc[c][:], PM[:], L_sb[c][:])

                # ---- numerators, divide, transpose -------------------------
                attnT = [dpool.tile([128, S_BLK], BF16, name=f"attnT{hp}")
                         for hp in range(NHP)]
                for c in range(NCH):
                    cs = slice(c * 128, (c + 1) * 128)
                    for hp in range(NHP):
                        ap_ = tpool.tile([128, 128], BF16, name="attnp")
                        for sub in range(2):
                            h = 2 * hp + sub
                            hb = slice(sub * 64, (sub + 1) * 64)
                            psN = ps.tile([128, HP], F32, name="psu", bufs=2,
                                          padded_shape=[128, 512])
                            nc.tensor.matmul(
                                psN[:],
                                am_sb[c][hp][:, sub * 128:(sub + 1) * 128],
                                v_tm[c][:, h * HP:(h + 1) * HP],
                                start=True, stop=False)
                            nc.tensor.matmul(
                                psN[:], sqT[hp][hb, cs],
                                PPc[c][hb, hp * HP:(hp + 1) * HP],
                                start=False, stop=True)
                            den = spool.tile([128, 1], F32, name="den")
                            nc.vector.tensor_scalar_add(
                                den[:], psN[:, DH:DH + 1], EPS)
                            rec = spool.tile([128, 1], F32, name="rec")
                            nc.vector.reciprocal(rec[:], den[:])
                            nc.vector.tensor_scalar_mul(
                                ap_[:, sub * DH:(sub + 1) * DH],
                                psN[:, 0:DH], rec[:])
                        psT = ps.tile([128, 128], BF16, name="pstr", bufs=2)
                        nc.tensor.transpose(psT[:], ap_[:], ident[:])
                        nc.vector.tensor_copy(attnT[hp][:, cs], psT[:])

                # ---- output projection -------------------------------------
                for st in range(NCH):
                    ss = slice(st * 128, (st + 1) * 128)
                    psO = ps.tile([128, D], F32, name="psbig", bufs=2)
                    for hp in range(NHP):
                        nc.tensor.matmul(psO[:], attnT[hp][:, ss], wo_t[hp][:],
                                         start=(hp == 0), stop=(hp == NHP - 1))
                    y_sb = tpool.tile([128, D], F32, name="ysb", bufs=2)
                    nc.vector.tensor_copy(y_sb[:], psO[:])
                    nc.sync.dma_start(y_d[ss, :], y_sb[:])

    nc.compile()
    return nc


def _get_nc():
    if "nc" not in _CACHE:
        _CACHE["nc"] = _build()
    return _CACHE["nc"]


def _make_in_maps(hidden_states, Wq, bq, Wk, bk, Wv, bv, Wo, M_mem, z_mem):
    hs = np.asarray(hidden_states, np.float32).reshape(S, D)
    Wq = np.ascontiguousarray(np.asarray(Wq, np.float32))
    Wk = np.ascontiguousarray(np.asarray(Wk, np.float32))
    Wv = np.ascontiguousarray(np.asarray(Wv, np.float32))
    Wo = np.ascontiguousarray(np.asarray(Wo, np.float32))
    bq = np.asarray(bq, np.float32)
    bk = np.asarray(bk, np.float32)
    bv = np.asarray(bv, np.float32)
    M_mem = np.asarray(M_mem, np.float32)
    z_mem = np.asarray(z_mem, np.float32)

    mz = np.zeros((128, W), np.float32)
    for h in range(H):
        pr, col = (h % 2) * 64, (h // 2) * HP
        mz[pr:pr + 64, col:col + DH] = M_mem[h]
        mz[pr:pr + 64, col + DH] = z_mem[h]

    in_maps = []
    for c in range(N_CORES):
        aux = np.zeros((128, AUXW), np.float32)
        aux[:, 0:4] = bq.reshape(NHP, 128).T
        aux[:, 4:8] = bk.reshape(NHP, 128).T
        aux[:, 8:12] = bv.reshape(NHP, 128).T
        aux[:, 12:12 + W] = mz
        aux[:, 12 + W:12 + W + c] = 1.0
        sidx = np.full(128, c, dtype=np.uint32)
        in_maps.append({
            "hs": np.ascontiguousarray(hs[c * S_BLK:(c + 1) * S_BLK]),
            "wq": Wq, "wk": Wk, "wv": Wv, "wo": Wo,
            "aux": aux,
            "sidx": sidx.reshape(128, 1),
        })
    return in_maps


def kernel(hidden_states, Wq, bq, Wk, bk, Wv, bv, Wo, M_mem, z_mem):
    nc = _get_nc()
    in_maps = _make_in_maps(hidden_states, Wq, bq, Wk, bk, Wv, bv, Wo,
                            M_mem, z_mem)
    res = run_bass_kernel_spmd(nc, in_maps, list(range(N_CORES)))
    out = np.concatenate([res.results[c]["y"] for c in range(N_CORES)], axis=0)
    return out.reshape(1, S, D)
